# revision 43
# baseline (speedup 1.0000x reference)
"""GCN layer (out = D^-1/2 (A+I) D^-1/2 (x W^T + b)) on 8 trn2 NeuronCores.

Strategy (v2, single device phase):
  Linearity refactor:
    out[dst] = rdeg_dst * ( (sum_{e->dst} rdeg_src * x_src) @ W^T
                            + (sum_{e->dst} rdeg_src) * b )
  - Host: append self-loops, sort edges by dst, group dst-blocks (128
    nodes) into slots of 8 (one block per core) by exact per-subtable
    chunk-count vector to minimize the uniform SPMD schedule's
    max-over-cores chunk counts. Supply x pre-scaled by rdeg as bf16
    subtables (int16-indexable), plus host-computed per-dst rdeg and
    rdeg-weighted-degree (s2) tables.
  - Device, per dst-block: batched dma_gather of scaled-x rows per edge
    chunk (128 edges), selection matrix via is_equal (dst_rel vs iota),
    PE matmul gt^T @ S accumulating agg^T = [in_ch, dst] into PSUM, then
    agg^T -> bf16, one W matmul + rank-1 s2*b bias matmul per block, and
    a scaled (rdeg_dst) drain to the output rows.
"""

import math
import time
from contextlib import ExitStack

import ml_dtypes
import numpy as np

import concourse.bass as bass
import concourse.tile as tile
from concourse import bacc, mybir
from concourse.bass_utils import run_bass_kernel_spmd

F32 = mybir.dt.float32
BF16 = mybir.dt.bfloat16
I16 = mybir.dt.int16

# ---------------------------------------------------------------------------
# Host-side planning
# ---------------------------------------------------------------------------


class Plan:
    pass


def build_plan(src_all, dst_all, n_nodes, n_cores, d=128,
               subt_cap=32768, gb=32, sg=8, n_loops=0):
    """src_all/dst_all: edge endpoints INCLUDING self loops (the final
    `n_loops` entries must be the appended self loops — those are served
    by a packed contiguous per-block load + identity matmuls instead of
    the per-row gather, halving their DMA cost).
    Returns Plan with static schedule + per-core data arrays."""
    t0 = time.time()
    p = Plan()
    p.d = d
    p.gb = gb          # chunks per gather batch
    p.sg = sg          # chunks per is_equal batch
    p.n_nodes = n_nodes
    p.n_cores = n_cores

    n_blocks_real = math.ceil(n_nodes / 128)
    slots = math.ceil(n_blocks_real / n_cores)
    p.slots = slots
    n_blocks = slots * n_cores
    p.n_pad = n_blocks * 128

    # sort the FULL stream (incl. self loops) by dst for degrees/bias
    order = np.argsort(dst_all, kind="stable")
    dst_s_full = np.asarray(dst_all)[order].astype(np.int64)
    src_s_full = np.asarray(src_all)[order].astype(np.int64)
    rowptr_full = np.searchsorted(dst_s_full, np.arange(p.n_pad + 1))

    # degrees (self-loops included); ghosts get deg 1
    deg = (rowptr_full[1:] - rowptr_full[:-1]).astype(np.float64)
    deg[n_nodes:] = 1.0
    deg[deg == 0] = 1.0
    rdeg = (1.0 / np.sqrt(deg)).astype(np.float32)  # [n_pad]
    p.rdeg = rdeg

    # rdeg-weighted in-degree per dst (bias coefficient)
    s2_g = np.zeros(p.n_pad, dtype=np.float32)
    np.add.at(s2_g, dst_s_full, rdeg[src_s_full])

    # the gather stream covers only the REAL edges; appended self loops
    # are handled by the packed self-row load in the device kernel
    n_real = len(np.asarray(src_all)) - n_loops
    src_r = np.asarray(src_all)[:n_real]
    dst_r = np.asarray(dst_all)[:n_real]
    order_r = np.argsort(dst_r, kind="stable")
    dst_s = dst_r[order_r].astype(np.int64)
    src_s = src_r[order_r].astype(np.int64)
    rowptr = np.searchsorted(dst_s, np.arange(p.n_pad + 1))

    blk_ptr = rowptr[::128]  # [n_blocks+1]
    blk_of_edge = (dst_s >> 7).astype(np.int64)

    def plan_subtables(sizes):
        """Per-(block, subtable) chunk counts + slot grouping for candidate
        subtable sizes. Groups blocks into slots of n_cores minimizing the
        summed per-q max-over-cores chunks: the uniform SPMD schedule
        executes that max on every core, so only the grouping matters (not
        which core gets which block). Bucket by exact per-q chunk vector
        (identical vectors in a group -> zero waste), then lexicographically
        matched leftovers."""
        off = np.cumsum([0] + list(sizes))
        nq = len(sizes)
        qs_all = np.searchsorted(off[1:], src_s, side="right")
        cnt = np.bincount(blk_of_edge * nq + qs_all,
                          minlength=n_blocks * nq).reshape(n_blocks, nq)
        C = (cnt + 127) // 128
        from collections import defaultdict
        buckets = defaultdict(list)
        for b in range(n_blocks):
            buckets[tuple(C[b])].append(b)
        groups = []
        leftover = []
        for v in sorted(buckets, reverse=True):
            blist = buckets[v]
            while len(blist) >= n_cores:
                groups.append(blist[:n_cores])
                blist = blist[n_cores:]
            leftover.extend(blist)
        leftover.sort(key=lambda b: tuple(C[b]), reverse=True)
        while leftover:
            g = leftover[:n_cores]
            leftover = leftover[n_cores:]
            while len(g) < n_cores:
                g.append(-1)  # ghost slot
            groups.append(g)
        while len(groups) < slots:
            groups.append([-1] * n_cores)
        assert len(groups) == slots, (len(groups), slots)

        def gcost(g):
            real = [b for b in g if b >= 0]
            return int(C[real].max(axis=0).sum()) if real else 0

        gcosts = [gcost(g) for g in groups]
        return sum(gcosts), groups, gcosts, C

    # search subtable splits (multiples of 128 rows, each <= subt_cap =
    # int16 idx range): uneven splits change where the per-(block,q) ceil
    # rounding lands, and total executed chunks with it
    n_subt = math.ceil(p.n_pad / subt_cap)
    base = (p.n_pad // n_subt) // 128 * 128
    cands = [[base] * (n_subt - 1) + [p.n_pad - base * (n_subt - 1)]]
    if n_subt > 1:
        for a in range(base, subt_cap + 1, 512):
            rem = p.n_pad - a * (n_subt - 1)
            if 0 < rem <= subt_cap:
                cands.append([a] * (n_subt - 1) + [rem])
    best = None

    def consider(sizes):
        nonlocal best
        if not all(0 < sz <= subt_cap and sz % 128 == 0 for sz in sizes):
            return
        r = plan_subtables(sizes)
        if best is None or r[0] < best[1][0]:
            best = (sizes, r)

    for sizes in cands:
        consider(sizes)
    if n_subt > 1:
        a0 = best[0][0]
        for da in (-256, -128, 128, 256):
            a = a0 + da
            rem = p.n_pad - a * (n_subt - 1)
            if 0 < rem <= subt_cap:
                consider([a] * (n_subt - 1) + [rem])
        # coordinate descent over fully independent sizes (last size is the
        # remainder): the per-(block,q) ceil landing points shift per size
        for _ in range(3):
            prev = best[0]
            for qi in range(n_subt - 1):
                for da in (-384, -256, -128, 128, 256, 384):
                    sz = list(best[0])
                    sz[qi] += da
                    sz[-1] = p.n_pad - sum(sz[:-1])
                    consider(sz)
            if best[0] == prev:
                break
    sizes, (_, groups, gcosts, blk_chunks) = best
    p.subt_sizes = sizes
    p.subt_off = np.cumsum([0] + list(sizes))  # [n_subt+1]
    p.n_subt = n_subt

    # per-block per-subtable edge lists (for the chosen sizes)
    subt_of_src = np.searchsorted(p.subt_off[1:], src_s, side="right")
    blk_edges = []  # per block: list per subt of (src_rel, dst_local) arrays
    for b in range(n_blocks):
        lo, hi = blk_ptr[b], blk_ptr[b + 1]
        qs = subt_of_src[lo:hi]
        per_q = []
        for q in range(n_subt):
            m = qs == q
            sl = src_s[lo:hi][m] - p.subt_off[q]
            dl = dst_s[lo:hi][m] - b * 128
            per_q.append((sl, dl))
        blk_edges.append(per_q)
    # largest groups first: the compute tail after the final gather is the
    # last slots' work, so keep the smallest there
    order_g = sorted(range(len(groups)), key=lambda i: -gcosts[i])
    groups = [groups[i] for i in order_g]
    core_blocks = [[int(groups[s][c]) for s in range(slots)]
                   for c in range(n_cores)]
    p.core_blocks = core_blocks  # [n_cores][slots] block id or -1

    # static schedule: k[s][q] = max over cores of chunks for that slot/subt
    k_sq = np.zeros((slots, n_subt), dtype=np.int64)
    for c in range(n_cores):
        for s in range(slots):
            b = core_blocks[c][s]
            if b >= 0:
                k_sq[s] = np.maximum(k_sq[s], blk_chunks[b])
    for s in range(slots):
        if k_sq[s].sum() == 0:
            k_sq[s, 0] = 1  # ensure psum gets start/stop
    p.k_sq = k_sq
    p.k_s = k_sq.sum(axis=1)  # chunks per slot
    n_chunks = int(p.k_s.sum())

    # chunk order (slot-major, subt groups in order) + per-subt stream pos
    chunk_q = np.zeros(n_chunks, dtype=np.int64)
    chunk_slot = np.zeros(n_chunks, dtype=np.int64)
    chunk_pos = np.zeros(n_chunks, dtype=np.int64)
    qcount = [0] * n_subt
    ci = 0
    for s in range(slots):
        for q in range(n_subt):
            for _ in range(int(k_sq[s, q])):
                chunk_q[ci] = q
                chunk_slot[ci] = s
                chunk_pos[ci] = qcount[q]
                qcount[q] += 1
                ci += 1
    assert ci == n_chunks
    p.n_chunks = n_chunks
    p.chunk_q = chunk_q
    p.chunk_slot = chunk_slot
    p.chunk_pos = chunk_pos
    p.qcounts = list(qcount)  # real chunks per subtable stream
    p.stream_len = [math.ceil(qcount[q] / gb) * gb if qcount[q] else 0
                    for q in range(n_subt)]   # in chunks, gb-aligned
    # gather batch tables (gb-chunk batches, final batch trimmed to the
    # real chunk count: gather cost is per descriptor = per row)
    p.batches = []   # per q: list of (start_chunk, n_chunks)
    p.pos2b = []     # per q: chunk stream pos -> batch index
    for q in range(n_subt):
        cnt = qcount[q]
        bl = []
        pos = 0
        while pos < cnt:
            n = min(gb, cnt - pos)
            bl.append((pos, n))
            pos += n
        p.batches.append(bl)
        lut = np.zeros(max(cnt, 1), dtype=np.int64)
        for bi, (st, n) in enumerate(bl):
            lut[st:st + n] = bi
        p.pos2b.append(lut)

    # per-core data tables
    p.core_idx = []      # per core: list per q of int16 [128, stream_len*8]
    p.core_dst_rel = []  # per core: bf16 [128, n_chunks]
    p.core_rdeg_s = []   # f32 [128, slots]
    p.core_s2 = []       # bf16 [1, slots*128]
    for c in range(n_cores):
        idx_q = [np.zeros((sl * 128,), dtype=np.int16) for sl in p.stream_len]
        dst_rel = np.full((128, n_chunks), -1.0, dtype=np.float32)
        rdeg_s = np.ones((128, slots), dtype=np.float32)
        s2 = np.zeros((1, slots * 128), dtype=np.float32)
        consumed = {}
        for ci in range(n_chunks):
            s, q, pos = int(chunk_slot[ci]), int(chunk_q[ci]), int(chunk_pos[ci])
            b = core_blocks[c][s]
            if b < 0:
                continue
            key = (s, q)
            off = consumed.get(key, 0)
            sl_arr, dl_arr = blk_edges[b][q]
            take = sl_arr[off:off + 128]
            if len(take):
                lanes = len(take)
                idx_q[q][pos * 128: pos * 128 + lanes] = take.astype(np.int16)
                dst_rel[:lanes, ci] = dl_arr[off:off + lanes]
            consumed[key] = off + 128
        for s in range(slots):
            b = core_blocks[c][s]
            if b >= 0:
                rdeg_s[:, s] = rdeg[b * 128: b * 128 + 128]
                s2[0, s * 128:(s + 1) * 128] = s2_g[b * 128: b * 128 + 128]
        # wrap idx into [128, len/16] layout (16-wrapped, replicated x8)
        idx_wrapped = []
        for q in range(n_subt):
            if p.stream_len[q] == 0:
                idx_wrapped.append(np.zeros((128, 1), dtype=np.int16))
                continue
            a = idx_q[q].reshape(-1, 16).T  # [16, n/16]
            idx_wrapped.append(np.tile(a, (8, 1)).copy())
        p.core_idx.append(idx_wrapped)
        p.core_dst_rel.append(dst_rel.astype(ml_dtypes.bfloat16))
        p.core_rdeg_s.append(rdeg_s)
        p.core_s2.append(s2.astype(ml_dtypes.bfloat16))

    p.plan_time = time.time() - t0
    return p


# ---------------------------------------------------------------------------
# Device kernel
# ---------------------------------------------------------------------------


def build_nc(p, n_cores=None, ob=8):
    """Build the uniform SPMD Bacc program for plan `p`.
    ob: output blocks per write."""
    d = p.d
    ob = min(ob, p.slots)

    nc = bacc.Bacc("TRN2", target_bir_lowering=False, debug=False,
                   num_devices=n_cores or p.n_cores)

    WT = nc.dram_tensor("WT", [d, d], BF16, kind="ExternalInput")
    bvec = nc.dram_tensor("bvec", [1, d], BF16, kind="ExternalInput")
    iota = nc.dram_tensor("iota", [128, 128], BF16, kind="ExternalInput")
    rdeg_s = nc.dram_tensor("rdeg_s", [128, p.slots], F32,
                            kind="ExternalInput")
    s2 = nc.dram_tensor("s2", [1, p.slots * 128], BF16, kind="ExternalInput")
    dst_rel = nc.dram_tensor("dst_rel", [128, p.n_chunks], BF16,
                             kind="ExternalInput")
    selfrows = nc.dram_tensor("selfrows", [p.slots * 128, d], BF16,
                              kind="ExternalInput")
    id0 = nc.dram_tensor("id0", [64, d], BF16, kind="ExternalInput")
    id1 = nc.dram_tensor("id1", [64, d], BF16, kind="ExternalInput")
    idx_t = []
    xq_t = []
    for q in range(p.n_subt):
        cols = max(p.stream_len[q] * 8, 1)  # 128 idx/chunk / 16
        idx_t.append(nc.dram_tensor(f"idx{q}", [128, cols], I16,
                                    kind="ExternalInput"))
        xq_t.append(nc.dram_tensor(f"xq{q}", [p.subt_sizes[q], d], BF16,
                                   kind="ExternalInput"))
    out_t = nc.dram_tensor("out", [p.slots * 128, d], F32,
                           kind="ExternalOutput")

    with tile.TileContext(nc) as tc, ExitStack() as ctx:
        # ---- constants (idx tables first: first gathers depend on them) ----
        cpool = ctx.enter_context(tc.tile_pool(name="consts", bufs=1))
        idx_sb = []
        for q in range(p.n_subt):
            t = cpool.tile([128, idx_t[q].shape[1]], I16, name=f"idxsb{q}")
            nc.sync.dma_start(t[:], idx_t[q].ap()[:, :])
            idx_sb.append(t)
        iota_sb = cpool.tile([128, 128], BF16)
        nc.sync.dma_start(iota_sb[:], iota.ap()[:, :])
        dstrel_sb = cpool.tile([128, p.n_chunks], BF16)
        nc.sync.dma_start(dstrel_sb[:], dst_rel.ap()[:, :])
        WT_sb = cpool.tile([d, d], BF16)
        nc.sync.dma_start(WT_sb[:], WT.ap()[:, :])
        b_sb = cpool.tile([1, d], BF16)
        nc.sync.dma_start(b_sb[:], bvec.ap()[:, :])
        rdeg_sb = cpool.tile([128, p.slots], F32)
        nc.sync.dma_start(rdeg_sb[:], rdeg_s.ap()[:, :])
        s2_sb = cpool.tile([1, p.slots * 128], BF16)
        nc.sync.dma_start(s2_sb[:], s2.ap()[:, :])
        id0_sb = cpool.tile([64, d], BF16)
        nc.sync.dma_start(id0_sb[:], id0.ap()[:, :])
        id1_sb = cpool.tile([64, d], BF16)
        nc.sync.dma_start(id1_sb[:], id1.ap()[:, :])
        # all self-loop rows in one packed load: row s*128 + p*2 + j lands
        # at partition p, column (s*2+j)*d — 512B contiguous per (p, s)
        # descriptor, so no sub-512B DMA penalty
        self_sb = cpool.tile([64, p.slots * 2 * d], BF16, name="selfsb")
        nc.sync.dma_start(
            out=self_sb[:].rearrange("p (s j d) -> p s j d", j=2, d=d),
            in_=selfrows.ap()[:, :].rearrange("(s p j) d -> p s j d",
                                              p=64, j=2))

        # ---- gather + aggregate + per-block W matmul ----
        gpools = [ctx.enter_context(
            tc.tile_pool(name=f"g{q}", bufs=2)) for q in range(p.n_subt)]
        stpool = ctx.enter_context(tc.tile_pool(name="st", bufs=6))
        aggpool = ctx.enter_context(tc.tile_pool(name="agg", bufs=4))
        psumA = ctx.enter_context(tc.tile_pool(name="psumA", bufs=6,
                                               space="PSUM"))
        psumB = ctx.enter_context(tc.tile_pool(name="psumB", bufs=2,
                                               space="PSUM"))
        opool = ctx.enter_context(tc.tile_pool(name="ostage", bufs=2))

        gtiles = {}

        def get_gtile(q, i):
            if (q, i) not in gtiles:
                st_c, n = p.batches[q][i]
                gt = gpools[q].tile([128, p.gb * d], BF16, name=f"gt{q}")
                nc.gpsimd.dma_gather(
                    out_ap=gt[:, 0:n * d].rearrange("p (j d) -> p j d", d=d),
                    in_ap=xq_t[q].ap()[:, :],
                    idxs_ap=idx_sb[q][:, st_c * 8:(st_c + n) * 8],
                    num_idxs=n * 128,
                    num_idxs_reg=n * 128,
                    elem_size=d,
                    single_packet=False,
                )
                gtiles[(q, i)] = gt
            return gtiles[(q, i)]

        st_tile = None
        ostage = None
        ci = 0
        for s in range(p.slots):
            ps_agg = psumA.tile([128, d], F32, name="ps_agg", space="PSUM")
            # self-loop term: two identity matmuls over the packed self rows
            # (partition p holds locals 2p / 2p+1 of the slot's block)
            so = s * 2 * d
            nc.tensor.matmul(out=ps_agg[:], lhsT=self_sb[:, so:so + d],
                             rhs=id0_sb[:], start=True, stop=False)
            nc.tensor.matmul(out=ps_agg[:], lhsT=self_sb[:, so + d:so + 2 * d],
                             rhs=id1_sb[:], start=False, stop=False)
            ks = int(p.k_s[s])
            for j in range(ks):
                if ci % p.sg == 0:
                    ng = min(p.sg, p.n_chunks - ci)
                    st_tile = stpool.tile([128, p.sg * 128], BF16, name="st_t")
                    nc.vector.tensor_tensor(
                        out=st_tile[:, 0:ng * 128].rearrange(
                            "p (g i) -> p g i", i=128),
                        in0=dstrel_sb[:, ci:ci + ng].unsqueeze(2).broadcast_to(
                            (128, ng, 128)),
                        in1=iota_sb[:].unsqueeze(1).broadcast_to((128, ng, 128)),
                        op=mybir.AluOpType.is_equal)
                q = int(p.chunk_q[ci])
                pos = int(p.chunk_pos[ci])
                bi = int(p.pos2b[q][pos])
                gt = get_gtile(q, bi)
                o = (pos - p.batches[q][bi][0]) * d
                stoff = (ci % p.sg) * 128
                # agg^T[in_ch, dst] += gt[e, in_ch]^T @ st[e, dst]
                nc.tensor.matmul(out=ps_agg[:], lhsT=gt[:, o:o + d],
                                 rhs=st_tile[:, stoff:stoff + 128],
                                 start=False, stop=(j == ks - 1))
                ci += 1
            aggT = aggpool.tile([128, d], BF16, name="aggT")
            nc.scalar.activation(aggT[:], ps_agg[:],
                                 mybir.ActivationFunctionType.Copy)
            out_ps = psumB.tile([128, d], F32, name="out_ps", space="PSUM")
            # out[dst, oc] = agg^T[ic, dst]^T @ W^T[ic, oc] + s2[dst] * b[oc]
            nc.tensor.matmul(out=out_ps[:], lhsT=aggT[:], rhs=WT_sb[:],
                             start=True, stop=False)
            nc.tensor.matmul(out=out_ps[:],
                             lhsT=s2_sb[:1, s * 128:(s + 1) * 128],
                             rhs=b_sb[:1, :], start=False, stop=True)
            if s % ob == 0:
                ostage = opool.tile([128, ob * d], F32, name="ostage")
                wstart = s  # first unflushed slot in this stage tile
            ocol = (s % ob) * d
            nc.scalar.activation(ostage[:, ocol:ocol + d], out_ps[:],
                                 mybir.ActivationFunctionType.Copy,
                                 scale=rdeg_sb[:, s:s + 1])
            # batched writes; in the last two groups flush every 2 slots so
            # the write chain after the final gather stays short
            group_end = (s % ob == ob - 1 or s == p.slots - 1)
            tailzone = s >= p.slots - 2 * ob
            if group_end or (tailzone and (s - wstart) % 2 == 1):
                nsw = s - wstart + 1
                c0 = (wstart % ob) * d
                nc.sync.dma_start(
                    out=out_t.ap()[wstart * 128:(wstart + nsw) * 128, :]
                    .rearrange("(j p) d -> p j d", p=128),
                    in_=ostage[:, c0:c0 + nsw * d].rearrange(
                        "p (j d) -> p j d", d=d))
                wstart = s + 1
        assert ci == p.n_chunks

    nc.compile()
    return nc


# ---------------------------------------------------------------------------
# Orchestration
# ---------------------------------------------------------------------------


def make_inputs(p, x, W, b):
    d = p.d
    xs = np.zeros((p.n_pad, d), dtype=np.float32)
    xs[:p.n_nodes] = np.asarray(x, dtype=np.float32)
    xs *= p.rdeg[:, None]
    xs_bf = xs.astype(ml_dtypes.bfloat16)
    WT = np.ascontiguousarray(np.asarray(W, dtype=np.float32).T).astype(
        ml_dtypes.bfloat16)
    bvec = np.asarray(b, dtype=np.float32).reshape(1, d).astype(
        ml_dtypes.bfloat16)
    iota = np.broadcast_to(np.arange(128, dtype=np.float32),
                           (128, 128)).astype(ml_dtypes.bfloat16).copy()
    id0 = np.zeros((64, d), dtype=np.float32)
    id0[np.arange(64), np.arange(64) * 2] = 1.0
    id1 = np.zeros((64, d), dtype=np.float32)
    id1[np.arange(64), np.arange(64) * 2 + 1] = 1.0
    common = {"WT": WT, "bvec": bvec, "iota": iota,
              "id0": id0.astype(ml_dtypes.bfloat16),
              "id1": id1.astype(ml_dtypes.bfloat16)}
    for q in range(p.n_subt):
        lo, hi = int(p.subt_off[q]), int(p.subt_off[q + 1])
        common[f"xq{q}"] = np.ascontiguousarray(xs_bf[lo:hi])
    in_maps = []
    for c in range(p.n_cores):
        m = dict(common)
        m["rdeg_s"] = p.core_rdeg_s[c]
        m["s2"] = p.core_s2[c]
        m["dst_rel"] = p.core_dst_rel[c]
        selfrows = np.zeros((p.slots * 128, d), dtype=ml_dtypes.bfloat16)
        for s, b in enumerate(p.core_blocks[c]):
            if b >= 0:
                selfrows[s * 128:(s + 1) * 128] = xs_bf[b * 128:(b + 1) * 128]
        m["selfrows"] = selfrows
        for q in range(p.n_subt):
            m[f"idx{q}"] = p.core_idx[c][q]
        in_maps.append(m)
    return in_maps


def assemble_output(p, results):
    out = np.zeros((p.n_nodes, p.d), dtype=np.float32)
    for c in range(p.n_cores):
        oc = results[c]["out"]
        for s, b in enumerate(p.core_blocks[c]):
            if b < 0 or b * 128 >= p.n_nodes:
                continue
            lo = b * 128
            hi = min(lo + 128, p.n_nodes)
            out[lo:hi] = oc[s * 128: s * 128 + (hi - lo)]
    return out


def gcn_forward(x, edge_index, W, b, n_cores=8, trace=False, **plan_kw):
    n = x.shape[0]
    src = np.asarray(edge_index[0])
    dst = np.asarray(edge_index[1])
    loop = np.arange(n, dtype=src.dtype)
    src_all = np.concatenate([src, loop])
    dst_all = np.concatenate([dst, loop])
    p = build_plan(src_all, dst_all, n, n_cores, d=W.shape[0], n_loops=n,
                   **plan_kw)
    nc = build_nc(p)
    in_maps = make_inputs(p, x, W, b)
    res = run_bass_kernel_spmd(nc, in_maps, core_ids=list(range(n_cores)),
                               trace=trace)
    out = assemble_output(p, [r for r in res.results])
    return out, p, res


# ---------------------------------------------------------------------------
# Harness entry point: full inputs in, full output out.
# ---------------------------------------------------------------------------

N_NODES = 100000
N_EDGES = 1600000
IN_CH = 128
OUT_CH = 128
N_CORES = 8


def kernel(x, edge_index, W, b):
    """GCN layer forward on 8 trn2 NeuronCores. Inputs as in setup_inputs()."""
    x = np.asarray(x, dtype=np.float32)
    edge_index = np.asarray(edge_index)
    W = np.asarray(W, dtype=np.float32)
    b = np.asarray(b, dtype=np.float32)
    out, _p, _res = gcn_forward(x, edge_index, W, b, n_cores=N_CORES)
    return out.astype(np.float32)
